# revision 4
# baseline (speedup 1.0000x reference)
"""AAM-softmax (ArcFace) loss + top-1 accuracy on 8 TRN2 NeuronCores.

Strategy (class/tensor parallel, per sharding hint):
  - Shard the C=100000 class dim across 8 cores (12500 classes each).
  - Per core: normalize its weight shard on device (norms via ones-matmul
    broadcast, rsqrt via ln+exp so the whole kernel uses ONE ACT table set),
    matmul bf16 cosine logits [1024 x 12500], fused exp+row-sum on ScalarE
    (scale arg = S/||x_i|| per row, so x never needs normalizing), row max of
    exp values on VectorE (for the argmax==label test), and the label-column
    margin correction via an indexed dma_gather of w[label] rows.
  - One tiny AllGather of per-row stats [1024 x 4]; every core redundantly
    finishes logsumexp -> loss and the accuracy comparison.

Host-side prep inside kernel() is layout only: shard, transpose, pad, cast
weights/x to bf16, and precompute per-core label index/ownership tensors.
"""

import math
import os
import sys

import numpy as np

sys.path.insert(0, "/opt/trn_rl_repo")

import ml_dtypes  # noqa: E402

import concourse.bass as bass  # noqa: E402
import concourse.mybir as mybir  # noqa: E402
import concourse.tile as tile  # noqa: E402
from concourse import bacc  # noqa: E402

P = 128
B, D, C, NC = 1024, 192, 100000, 8
CL = C // NC  # classes per core
MT = B // P  # M tiles (batch rows / 128)
KP = 2  # contraction planes (D=192 padded to 2x128)
G_MAIN = 1536  # main psum group (3 banks), x2 bufs
G_NORM = 1024  # norm psum group (2 banks), x1 buf
CHUNK = 2048  # weight DMA/square chunk

M_MARG = 0.2
S_SCALE = 30.0
COS_M = math.cos(M_MARG)
SIN_M = math.sin(M_MARG)
TH = math.cos(math.pi - M_MARG)
MM = math.sin(math.pi - M_MARG) * M_MARG

BF16 = ml_dtypes.bfloat16


def _ranges(total, step):
    return [(o, min(step, total - o)) for o in range(0, total, step)]


def build_nc():
    nc = bacc.Bacc(
        "TRN2", target_bir_lowering=False, debug=False, num_devices=NC
    )
    f32 = mybir.dt.float32
    bf16 = mybir.dt.bfloat16
    i16 = mybir.dt.int16
    AX = mybir.AxisListType
    AF = mybir.ActivationFunctionType
    AL = mybir.AluOpType

    wt_d = nc.dram_tensor("wt", [P, KP, CL], bf16, kind="ExternalInput")
    xt_d = nc.dram_tensor("xt", [P, KP, B], bf16, kind="ExternalInput")
    xn_d = nc.dram_tensor("xnat", [P, MT, D], f32, kind="ExternalInput")
    wg_d = nc.dram_tensor("wnat", [CL, D], f32, kind="ExternalInput")
    ix_d = nc.dram_tensor("idx", [P, B // 16], i16, kind="ExternalInput")
    ow_d = nc.dram_tensor("own", [P, MT], f32, kind="ExternalInput")
    out_d = nc.dram_tensor("out", [1, 2], f32, kind="ExternalOutput")

    chunks = _ranges(CL, CHUNK)
    norm_groups = _ranges(CL, G_NORM)
    main_groups = _ranges(CL, G_MAIN)

    with tile.TileContext(nc) as tc:
        with (
            tc.tile_pool(name="wtp", bufs=3) as wtp,
            tc.tile_pool(name="wnp", bufs=1) as wnp,
            tc.tile_pool(name="sqp", bufs=2) as sqp,
            tc.tile_pool(name="rwp", bufs=2) as rwp,
            tc.tile_pool(name="lnp", bufs=2) as lnp,
            tc.tile_pool(name="expp", bufs=2) as expp,
            tc.tile_pool(name="smp", bufs=1) as smp,
            tc.tile_pool(name="mps", bufs=2, space="PSUM") as mps,
            tc.tile_pool(name="nps", bufs=1, space="PSUM") as nps,
            tc.tile_pool(name="drp", bufs=1, space="DRAM") as drp,
        ):
            # ---------------- input DMAs ----------------
            xt = smp.tile([P, KP, B], bf16, tag="xt")
            nc.sync.dma_start(out=xt[:], in_=xt_d[:])
            xnat = smp.tile([P, MT, D], f32, tag="xnat")
            nc.sync.dma_start(out=xnat[:], in_=xn_d[:])
            own = smp.tile([P, MT], f32, tag="own")
            nc.sync.dma_start(out=own[:], in_=ow_d[:])
            idx = smp.tile([P, B // 16], i16, tag="idx")
            nc.sync.dma_start(out=idx[:], in_=ix_d[:])

            ones_bf = smp.tile([P, P], bf16, tag="onesbf")
            nc.vector.memset(ones_bf[:], 1.0)
            ones_f = smp.tile([P, 1], f32, tag="onesf")
            nc.vector.memset(ones_f[:], 1.0)
            bias_one = smp.tile([P, 1], f32, tag="bias_one")
            nc.vector.memset(bias_one[:], 1.0)
            bias_eps = smp.tile([P, 1], f32, tag="bias_eps")
            nc.vector.memset(bias_eps[:], 1e-37)

            wt_c = []
            for ci, (coff, csz) in enumerate(chunks):
                t = wtp.tile([P, KP, csz], bf16, tag="wt")
                nc.sync.dma_start(out=t[:], in_=wt_d[:, :, coff : coff + csz])
                wt_c.append(t)

            # ---------------- x norms: rxS = S / ||x_i|| ----------------
            xsq = smp.tile([P, MT, D], f32, tag="xsq")
            nc.vector.tensor_tensor(out=xsq[:], in0=xnat[:], in1=xnat[:], op=AL.mult)
            n2x = smp.tile([P, MT], f32, tag="n2x")
            nc.vector.tensor_reduce(out=n2x[:], in_=xsq[:], axis=AX.X, op=AL.add)
            lnx = smp.tile([P, MT], f32, tag="lnx")
            nc.scalar.activation(lnx[:], n2x[:], AF.Ln)
            rx = smp.tile([P, MT], f32, tag="rx")
            nc.scalar.activation(rx[:], lnx[:], AF.Exp, scale=-0.5)
            rxS = smp.tile([P, MT], f32, tag="rxS")
            nc.vector.tensor_scalar_mul(rxS[:], rx[:], S_SCALE)

            # ---------------- weight prep: wn = w / ||w_j|| ----------------
            wn_g = []
            for gi, (goff, gsz) in enumerate(norm_groups):
                ci = goff // CHUNK
                soff = goff - chunks[ci][0]
                wtc = wt_c[ci]
                sq = sqp.tile([P, KP, G_NORM], bf16, tag="sq")
                nc.vector.tensor_tensor(
                    out=sq[:, :, 0:gsz],
                    in0=wtc[:, :, soff : soff + gsz],
                    in1=wtc[:, :, soff : soff + gsz],
                    op=AL.mult,
                )
                nps_t = nps.tile([P, G_NORM], f32, tag="norm")
                for so, ss in _ranges(gsz, 512):
                    for k in range(KP):
                        nc.tensor.matmul(
                            nps_t[:, so : so + ss],
                            ones_bf[:, 0:P],
                            sq[:, k, so : so + ss],
                            start=(k == 0),
                            stop=(k == KP - 1),
                        )
                lnn = lnp.tile([P, G_NORM], f32, tag="ln")
                nc.scalar.activation(lnn[:, 0:gsz], nps_t[:, 0:gsz], AF.Ln)
                rwb = rwp.tile([P, G_NORM], bf16, tag="rw")
                nc.scalar.activation(rwb[:, 0:gsz], lnn[:, 0:gsz], AF.Exp, scale=-0.5)
                wn = wnp.tile([P, KP, gsz], bf16, tag=f"wn{gi}")
                for k in range(KP):
                    nc.vector.tensor_tensor(
                        out=wn[:, k, :],
                        in0=wtc[:, k, soff : soff + gsz],
                        in1=rwb[:, 0:gsz],
                        op=AL.mult,
                    )
                wn_g.append(wn)

            # ---------------- main: logits -> exp -> sum/max ----------------
            NGM = len(main_groups)
            acc = smp.tile([P, MT, NGM], f32, tag="acc")
            val8 = smp.tile([P, MT, 10], f32, tag="val8")
            for m in range(MT):
                e_m = expp.tile([P, CL], bf16, tag="exp")
                for gi, (goff, gsz) in enumerate(main_groups):
                    ps = mps.tile([P, G_MAIN], f32, tag="main")
                    for k in range(KP):
                        for so, ss in _ranges(gsz, 512):
                            ngi = (goff + so) // G_NORM
                            woff = goff + so - ngi * G_NORM
                            nc.tensor.matmul(
                                ps[:, so : so + ss],
                                xt[:, k, m * P : (m + 1) * P],
                                wn_g[ngi][:, k, woff : woff + ss],
                                start=(k == 0),
                                stop=(k == KP - 1),
                            )
                    nc.scalar.activation(
                        e_m[:, goff : goff + gsz],
                        ps[:, 0:gsz],
                        AF.Exp,
                        scale=rxS[:, m : m + 1],
                        accum_out=acc[:, m, gi : gi + 1],
                    )
                nc.vector.tensor_reduce(
                    out=val8[:, m, :],
                    in_=e_m[:].rearrange("p (a b) -> p a b", a=10),
                    axis=AX.X,
                    op=AL.max,
                )

            # ---------------- small side: label margin path ----------------
            gw = smp.tile([P, MT, D], f32, tag="gw")
            nc.gpsimd.dma_gather(
                out_ap=gw[:],
                in_ap=wg_d[:],
                idxs_ap=idx[:],
                num_idxs=B,
                num_idxs_reg=B,
                elem_size=D,
            )
            tmp = smp.tile([P, MT, D], f32, tag="tmp")
            nc.vector.tensor_tensor(out=tmp[:], in0=gw[:], in1=xnat[:], op=AL.mult)
            ut = smp.tile([P, MT], f32, tag="ut")
            nc.vector.tensor_reduce(out=ut[:], in_=tmp[:], axis=AX.X, op=AL.add)
            nc.vector.tensor_tensor(out=tmp[:], in0=gw[:], in1=gw[:], op=AL.mult)
            n2t = smp.tile([P, MT], f32, tag="n2t")
            nc.vector.tensor_reduce(out=n2t[:], in_=tmp[:], axis=AX.X, op=AL.add)
            lnt = smp.tile([P, MT], f32, tag="lnt")
            nc.scalar.activation(lnt[:], n2t[:], AF.Ln)
            rwt = smp.tile([P, MT], f32, tag="rwt")
            nc.scalar.activation(rwt[:], lnt[:], AF.Exp, scale=-0.5)
            cost = smp.tile([P, MT], f32, tag="cost")
            nc.vector.tensor_tensor(out=cost[:], in0=ut[:], in1=rx[:], op=AL.mult)
            nc.vector.tensor_tensor(out=cost[:], in0=cost[:], in1=rwt[:], op=AL.mult)
            csq = smp.tile([P, MT], f32, tag="csq")
            nc.vector.tensor_tensor(out=csq[:], in0=cost[:], in1=cost[:], op=AL.mult)
            yrel = smp.tile([P, MT], f32, tag="yrel")
            nc.scalar.activation(yrel[:], csq[:], AF.Relu, scale=-1.0, bias=bias_one[:])
            lny = smp.tile([P, MT], f32, tag="lny")
            nc.scalar.activation(lny[:], yrel[:], AF.Ln, bias=bias_eps[:])
            sint = smp.tile([P, MT], f32, tag="sint")
            nc.scalar.activation(sint[:], lny[:], AF.Exp, scale=0.5)
            # phi = cos>TH ? cos*COS_M - sin*SIN_M : cos - MM
            pa = smp.tile([P, MT], f32, tag="pa")
            nc.vector.tensor_scalar_mul(pa[:], cost[:], COS_M)
            sb = smp.tile([P, MT], f32, tag="sb")
            nc.vector.tensor_scalar_mul(sb[:], sint[:], SIN_M)
            nc.vector.tensor_tensor(out=pa[:], in0=pa[:], in1=sb[:], op=AL.subtract)
            pb = smp.tile([P, MT], f32, tag="pb")
            nc.vector.tensor_scalar_sub(pb[:], cost[:], MM)
            mk = smp.tile([P, MT], mybir.dt.uint8, tag="mk")
            nc.vector.tensor_scalar(mk[:], cost[:], TH, None, AL.is_gt)
            phi = smp.tile([P, MT], f32, tag="phi")
            nc.vector.select(phi[:], mk[:], pa[:], pb[:])
            t_own = smp.tile([P, MT], f32, tag="t_own")
            nc.vector.tensor_tensor(out=t_own[:], in0=phi[:], in1=own[:], op=AL.mult)
            nc.vector.tensor_scalar_mul(t_own[:], t_own[:], S_SCALE)
            e_phi = smp.tile([P, MT], f32, tag="e_phi")
            nc.scalar.activation(e_phi[:], phi[:], AF.Exp, scale=S_SCALE)
            e_raw = smp.tile([P, MT], f32, tag="e_raw")
            nc.scalar.activation(e_raw[:], cost[:], AF.Exp, scale=S_SCALE)
            dcor = smp.tile([P, MT], f32, tag="dcor")
            nc.vector.tensor_tensor(out=dcor[:], in0=e_phi[:], in1=e_raw[:], op=AL.subtract)
            nc.vector.tensor_tensor(out=dcor[:], in0=dcor[:], in1=own[:], op=AL.mult)
            epo = smp.tile([P, MT], f32, tag="epo")
            nc.vector.tensor_tensor(out=epo[:], in0=e_phi[:], in1=own[:], op=AL.mult)

            # ---------------- assemble per-row stats ----------------
            sl = smp.tile([P, MT], f32, tag="sl")
            nc.vector.tensor_reduce(out=sl[:], in_=acc[:], axis=AX.X, op=AL.add)
            scorr = smp.tile([P, MT], f32, tag="scorr")
            nc.vector.tensor_tensor(out=scorr[:], in0=sl[:], in1=dcor[:], op=AL.add)
            val1l = smp.tile([P, MT], f32, tag="val1l")
            nc.vector.tensor_reduce(out=val1l[:], in_=val8[:], axis=AX.X, op=AL.max)

            st = smp.tile([P, 4, MT], f32, tag="st")
            nc.vector.tensor_copy(st[:, 0, :], scorr[:])
            nc.vector.tensor_copy(st[:, 1, :], t_own[:])
            nc.vector.tensor_copy(st[:, 2, :], epo[:])
            nc.vector.tensor_copy(st[:, 3, :], val1l[:])

            # ---------------- AllGather stats ----------------
            cc_in = drp.tile([P, 4 * MT], f32, tag="ccin")
            nc.sync.dma_start(out=cc_in[:], in_=st[:])
            cc_out = drp.tile([NC, P, 4 * MT], f32, tag="ccout", addr_space="Shared")
            nc.gpsimd.collective_compute(
                "AllGather",
                AL.bypass,
                replica_groups=[list(range(NC))],
                ins=[cc_in[:].opt()],
                outs=[cc_out[:].opt()],
            )
            gst = smp.tile([P, NC, 4, MT], f32, tag="gst")
            nc.sync.dma_start(
                out=gst[:], in_=cc_out[:].rearrange("c p f -> p c f")
            )

            # ---------------- combine + finish ----------------
            stot = smp.tile([P, MT], f32, tag="stot")
            ttot = smp.tile([P, MT], f32, tag="ttot")
            ephit = smp.tile([P, MT], f32, tag="ephit")
            val1g = smp.tile([P, MT], f32, tag="val1g")
            for ch, (dst, op) in enumerate(
                [(stot, AL.add), (ttot, AL.add), (ephit, AL.add), (val1g, AL.max)]
            ):
                nc.vector.tensor_reduce(
                    out=dst[:],
                    in_=gst[:, :, ch, :].rearrange("p c m -> p m c"),
                    axis=AX.X,
                    op=op,
                )
            lnS = smp.tile([P, MT], f32, tag="lnS")
            nc.scalar.activation(lnS[:], stot[:], AF.Ln)
            lossi = smp.tile([P, MT], f32, tag="lossi")
            nc.vector.tensor_tensor(out=lossi[:], in0=lnS[:], in1=ttot[:], op=AL.subtract)
            eq = smp.tile([P, MT], f32, tag="eq")
            nc.vector.tensor_tensor(out=eq[:], in0=ephit[:], in1=val1g[:], op=AL.is_ge)
            pair = smp.tile([P, 2], f32, tag="pair")
            nc.vector.tensor_reduce(out=pair[:, 0:1], in_=lossi[:], axis=AX.X, op=AL.add)
            nc.vector.tensor_reduce(out=pair[:, 1:2], in_=eq[:], axis=AX.X, op=AL.add)
            fin = nps.tile([P, G_NORM], f32, tag="norm")
            nc.tensor.matmul(fin[0:1, 0:2], ones_f[:, 0:1], pair[:, 0:2])
            consts = smp.tile([1, 2], f32, tag="consts")
            nc.vector.memset(consts[0:1, 0:1], 1.0 / B)
            nc.vector.memset(consts[0:1, 1:2], 100.0 / B)
            res = smp.tile([1, 2], f32, tag="res")
            nc.vector.tensor_tensor(
                out=res[0:1, :], in0=fin[0:1, 0:2], in1=consts[0:1, :], op=AL.mult
            )
            nc.sync.dma_start(out=out_d[:], in_=res[:])

    nc.compile()
    return nc


def make_in_maps(x, weight, label):
    x = np.asarray(x, dtype=np.float32)
    weight = np.asarray(weight, dtype=np.float32)
    label = np.asarray(label).astype(np.int64)

    xT = np.ascontiguousarray(x.T)  # [D, B] f32
    xt_p = np.zeros((P, KP, B), dtype=BF16)
    xt_p[:, 0, :] = xT[0:P].astype(BF16)
    xt_p[0 : D - P, 1, :] = xT[P:D].astype(BF16)
    x_nat = np.ascontiguousarray(x.reshape(MT, P, D).transpose(1, 0, 2))

    in_maps = []
    for c in range(NC):
        wb = weight[c * CL : (c + 1) * CL]  # [CL, D] f32
        wT = wb.T.astype(BF16)  # [D, CL]
        wt_p = np.zeros((P, KP, CL), dtype=BF16)
        wt_p[:, 0, :] = wT[0:P]
        wt_p[0 : D - P, 1, :] = wT[P:D]

        lab_loc = label - c * CL
        own = (lab_loc >= 0) & (lab_loc < CL)
        clamped = np.where(own, lab_loc, 0).astype(np.int16)
        idx_p = np.zeros((P, B // 16), dtype=np.int16)
        idx_p[0:16, :] = clamped.reshape(B // 16, 16).T
        own_p = np.ascontiguousarray(own.reshape(MT, P).T).astype(np.float32)

        in_maps.append(
            {
                "wt": wt_p,
                "xt": xt_p,
                "xnat": x_nat,
                "wnat": np.ascontiguousarray(wb),
                "idx": idx_p,
                "own": own_p,
            }
        )
    return in_maps


_CACHE = {}


def kernel(x, weight, label):
    from concourse.bass_utils import run_bass_kernel_spmd
    from concourse.bass_interp import get_hw_module

    if "nc" not in _CACHE:
        _CACHE["nc"] = build_nc()
    nc = _CACHE["nc"]

    in_maps = make_in_maps(x, weight, label)

    old_m = nc.m
    nc.m = get_hw_module(nc.m)
    try:
        r = run_bass_kernel_spmd(
            nc,
            in_maps,
            core_ids=list(range(NC)),
            trace=bool(int(os.environ.get("KERNEL_TRACE", "0"))),
        )
    finally:
        nc.m = old_m
    _CACHE["last_result"] = r

    out = r.results[0]["out"]
    loss = np.float32(out[0, 0])
    prec1 = np.float32(out[0, 1])
    return loss, prec1
